# revision 1
# baseline (speedup 1.0000x reference)
"""AWQ W4A16 Linear (out = x @ dequant(qweight) + bias) on 8 TRN2 NeuronCores.

Tensor-parallel over out_features: each core owns a contiguous slice of
N = 12288 (1536 columns), dequantizes its int4 weight shard on-chip into a
SBUF-resident bf16 [K, N_local] matrix, and runs a PE-bound matmul over the
replicated activations. No collectives; the host concatenates the 8 column
slices.

Per-core pipeline (single Bass program, graduated k-chunks CH_KT so the PE
can start while dequant is still running):
  dequant (DVE):   qweight int32 [384, i-chunk] -> fused (shift & 0xF)
                   tensor_scalar -> (q - z) exact in int -> * scale (bf16,
                   parity-strided write) -> W_pre rows staged to per-chunk
                   DRAM scratch (row n = 4t + r written n-naturally)
  W' load (SP):    one dma_start_transpose per chunk:
                   [1536 n, k-chunk] -> [128 k, kt, 1536 n], SBUF-resident
  x stream (ACT):  one xbar-transpose DMA per 512-row panel:
                   [512 m, 4096 k] -> [128 k, 32 kt, 512 m], prefetch depth 1
  matmul (PE):     psum [128 m, 512 n] over 32 k-tiles; kt-outer/nb-inner so
                   consecutive matmuls share the stationary lhsT; 8 psum banks
  evict:           ACT copy psum->bf16 (rounds once, matching the reference),
                   DVE adds bias, contiguous store per m-block.
"""

import numpy as np
import ml_dtypes
from contextlib import ExitStack

import concourse.bass as bass
import concourse.bacc as bacc
import concourse.mybir as mybir
import concourse.tile as tile
from concourse.bass_utils import run_bass_kernel_spmd

BF16 = mybir.dt.bfloat16
I32 = mybir.dt.int32
F32 = mybir.dt.float32

M_FULL = 4096
K = 4096
N_FULL = 12288
N_CORES = 8
NL = N_FULL // N_CORES          # 1536 out features per core
GS = 64                         # quant group size
NG = K // GS                    # 64 groups
NKT = K // 128                  # 32 k-tiles
PANEL = 512                     # m-panel size
NB = NL // 512                  # 3 psum n-blocks per core
# dequant k-chunks (pipelines W' availability); graduated so the first
# W' tiles reach the PE quickly: sizes are in k-tiles (128 k each)
CH_KT = [2, 2, 4, 8, 8, 8]
NCH = len(CH_KT)
CH_KT0 = [sum(CH_KT[:i]) for i in range(NCH)]   # start k-tile per chunk


def build_nc(m: int = M_FULL, phases: str = "all",
             repeat: int = 1, debug_taps: bool = False) -> bass.Bass:
    nc = bacc.Bacc(None)
    x = nc.dram_tensor("x", [m, K], BF16, kind="ExternalInput")
    qw = nc.dram_tensor("qw", [NL // 4, K // 2], I32, kind="ExternalInput")
    # st/zt[t, r, g] = wscales/wzeros[g, n0 + 4*t + r]  (host pre-arranged)
    st = nc.dram_tensor("st", [NL // 4, 4, NG], BF16, kind="ExternalInput")
    zt = nc.dram_tensor("zt", [NL // 4, 4, NG], I32, kind="ExternalInput")
    bias = nc.dram_tensor("bias", [NL], BF16, kind="ExternalInput")
    out = nc.dram_tensor("out", [m, NL], BF16, kind="ExternalOutput")

    n_mp = m // PANEL
    n_ms = PANEL // 128

    with tile.TileContext(nc) as tc, ExitStack() as ctx:
        dram = ctx.enter_context(tc.tile_pool(name="dram", bufs=1, space="DRAM"))
        # one DRAM staging tile per k-chunk so Tile's per-tile dependency
        # tracking lets W' k-tiles of finished chunks load early
        wpre_ch = []
        for ch in range(NCH):
            w_c = dram.tile([NL, CH_KT[ch] * 128], BF16, name=f"wpre{ch}",
                            uniquify=False)
            wpre_ch.append(w_c)

        const = ctx.enter_context(tc.tile_pool(name="const", bufs=1))
        bias_sb = const.tile([128, NL], BF16)
        bias_bc = bass.AP(
            tensor=bias[:].tensor, offset=bias[:].offset, ap=[[0, 128], [1, NL]]
        )
        nc.gpsimd.dma_start(out=bias_sb[:], in_=bias_bc)

        st_all = const.tile([128, 3, 4, NG], BF16)
        zt_all = const.tile([128, 3, 4, NG], I32)
        nc.sync.dma_start(
            out=st_all[:], in_=st[:].rearrange("(t3 p) r g -> p t3 r g", p=128))
        nc.sync.dma_start(
            out=zt_all[:], in_=zt[:].rearrange("(t3 p) r g -> p t3 r g", p=128))
        st_sb = [st_all[:, t3] for t3 in range(3)]
        zt_sb = [zt_all[:, t3] for t3 in range(3)]

        # ---- pipeline body ----
        for rep in range(repeat):
            _build_pipeline(nc, tc, qw, x, out, wpre_ch, st_sb, zt_sb, bias_sb,
                            m, n_mp, n_ms, phases)
    nc.compile()
    return nc


def _build_pipeline(nc, tc, qw, x, out, wpre_ch, st_sb, zt_sb, bias_sb,
                    m, n_mp, n_ms, phases):
      with ExitStack() as ctx:
        deq = ctx.enter_context(tc.tile_pool(name="deq", bufs=2))
        qwp = ctx.enter_context(tc.tile_pool(name="qwp", bufs=1))
        wprep = ctx.enter_context(tc.tile_pool(name="wprep", bufs=2))
        wqp = ctx.enter_context(tc.tile_pool(name="wqp", bufs=1))
        xpp = ctx.enter_context(tc.tile_pool(name="xpp", bufs=2))
        psp = ctx.enter_context(tc.tile_pool(name="psp", bufs=8, space="PSUM"))
        outp = ctx.enter_context(tc.tile_pool(name="outp", bufs=2))

        do_deq = phases in ("all", "deq")
        do_mm = phases in ("all", "mm")

        # x panel transpose-loads: issued on the ACT HWDGE ring so they are
        # not stuck behind the dequant staging traffic on the SP ring.
        # Only panel 0 is queued upfront; panel i+1 is queued when panel i's
        # matmuls are emitted, so early x traffic doesn't delay the first
        # W' chunk on the shared DMA engines.
        xp_tiles = []

        def load_panel(mp):
            xp_t = xpp.tile([128, NKT, PANEL], BF16, tag="xp", name=f"xp{mp}")
            # whole panel in one xbar-transpose DMA:
            # [PANEL, K] -> [128, NKT, PANEL] (out[:, e, :] = cols 128e..)
            nc.scalar.dma_start(
                out=xp_t[:],
                in_=x[mp * PANEL:(mp + 1) * PANEL, :],
                transpose=True,
            )
            xp_tiles.append(xp_t)

        if do_mm:
            load_panel(0)

        wq = [None] * NKT
        if do_deq:
            for ch in range(NCH):
                ich = CH_KT[ch] * 64        # packed int32 cols in this chunk
                i0 = CH_KT0[ch] * 64
                gch = CH_KT[ch] * 2         # 64-k groups in this chunk
                g0 = CH_KT0[ch] * 2
                # qweight shard k-chunk in one DMA: [384, ich] -> [128, 3, ich]
                qw_full = qwp.tile([128, 3, max(CH_KT) * 64], I32, tag="qw",
                                   name=f"qwb{ch}")
                qw_big = qw_full[:, :, :ich]
                nc.sync.dma_start(
                    out=qw_big,
                    in_=qw[:, i0:i0 + ich].rearrange("(t3 p) i -> p t3 i", p=128),
                )
                # wpre_ch[ch] viewed so row n = 4*t + r is addressed [r, t]
                w_rt = wpre_ch[ch][:].rearrange("(t four) k -> four t k", four=4)
                for r in range(4):
                    for t3 in range(3):
                        wp_full = wprep.tile([128, max(CH_KT) * 128], BF16,
                                             tag="wp", name=f"wp{ch}_{r}_{t3}")
                        wp_t = wp_full[:, :CH_KT[ch] * 128]
                        for c in range(2):
                            j = 2 * r + c
                            nib_full = deq.tile([128, max(CH_KT) * 64], I32,
                                                tag="nib", name=f"nib{ch}_{j}")
                            nib = nib_full[:, :ich]
                            nc.vector.tensor_scalar(
                                nib,
                                qw_big[:, t3, :],
                                4 * j, 0xF,
                                mybir.AluOpType.logical_shift_right,
                                mybir.AluOpType.bitwise_and,
                            )
                            diff_full = deq.tile([128, max(CH_KT) * 64], BF16,
                                                 tag="diff", name=f"diff{ch}_{j}")
                            diff = diff_full[:, :ich]
                            nib_g = nib.rearrange("p (g q) -> p g q", q=GS // 2)
                            diff_g = diff.rearrange("p (g q) -> p g q", q=GS // 2)
                            z_bc = zt_sb[t3][
                                :, r, g0:g0 + gch, None
                            ].broadcast_to([128, gch, GS // 2])
                            s_bc = st_sb[t3][
                                :, r, g0:g0 + gch, None
                            ].broadcast_to([128, gch, GS // 2])
                            nc.vector.tensor_tensor(
                                diff_g, nib_g, z_bc, mybir.AluOpType.subtract
                            )
                            # k_local = GS*g + 2*u + c
                            wp_view = wp_t.rearrange(
                                "p (g u two) -> p two g u", two=2, u=GS // 2
                            )[:, c]
                            nc.vector.tensor_tensor(
                                wp_view, diff_g, s_bc, mybir.AluOpType.mult
                            )
                        nc.sync.dma_start(
                            out=w_rt[r, t3 * 128:(t3 + 1) * 128], in_=wp_t
                        )
                if do_mm:
                    # all W' k-tiles of this chunk in ONE transpose DMA,
                    # right behind the chunk's stores on the SP ring
                    w_t = wqp.tile([128, CH_KT[ch], NL], BF16, tag=f"wq{ch}",
                                   name=f"wqc{ch}")
                    nc.sync.dma_start(
                        out=w_t[:], in_=wpre_ch[ch][:], transpose=True
                    )
                    for kt in range(CH_KT0[ch], CH_KT0[ch] + CH_KT[ch]):
                        wq[kt] = w_t[:, kt - CH_KT0[ch]]
        elif do_mm:
            for ch in range(NCH):
                w_t = wqp.tile([128, CH_KT[ch], NL], BF16, tag=f"wq{ch}",
                               name=f"wqc{ch}")
                nc.sync.dma_start(
                    out=w_t[:], in_=wpre_ch[ch][:], transpose=True
                )
                for kt in range(CH_KT0[ch], CH_KT0[ch] + CH_KT[ch]):
                    wq[kt] = w_t[:, kt - CH_KT0[ch]]

        if not do_mm:
            return
        for mp in range(n_mp):
            if mp + 1 < n_mp:
                load_panel(mp + 1)
            xp_t = xp_tiles[mp]
            for ms in range(n_ms):
                out_t = outp.tile([128, NL], BF16, tag="out")
                pss = [psp.tile([128, 512], F32, tag="ps", name=f"ps{nb}")
                       for nb in range(NB)]
                # kt outer / nb inner: 3 consecutive matmuls share the same
                # stationary lhsT (the PE skips redundant weight reloads)
                for kt in range(NKT):
                    for nb in range(NB):
                        nc.tensor.matmul(
                            pss[nb][:],
                            lhsT=xp_t[:, kt, ms * 128:(ms + 1) * 128],
                            rhs=wq[kt][:, nb * 512:(nb + 1) * 512],
                            start=(kt == 0),
                            stop=(kt == NKT - 1),
                        )
                for nb in range(NB):
                    o_slice = out_t[:, nb * 512:(nb + 1) * 512]
                    # psum -> sbuf bf16 cast on the (otherwise idle) ACT engine
                    nc.scalar.activation(
                        o_slice, pss[nb][:], mybir.ActivationFunctionType.Copy
                    )
                    nc.vector.tensor_tensor(
                        o_slice, o_slice,
                        bias_sb[:, nb * 512:(nb + 1) * 512],
                        mybir.AluOpType.add,
                    )
                nc.sync.dma_start(
                    out=out[mp * PANEL + ms * 128:mp * PANEL + (ms + 1) * 128],
                    in_=out_t[:],
                )


def shard_inputs(x, qweight, wscales, wzeros, bias):
    """Split the full problem into per-core input maps."""
    in_maps = []
    x = np.ascontiguousarray(x)
    for i in range(N_CORES):
        n0 = i * NL
        qw_s = np.ascontiguousarray(qweight[n0 // 4:(n0 + NL) // 4])
        s_s = np.ascontiguousarray(wscales[:, n0:n0 + NL].T).reshape(NL // 4, 4, NG)
        z_s = np.ascontiguousarray(
            wzeros[:, n0:n0 + NL].T.astype(np.int32)).reshape(NL // 4, 4, NG)
        b_s = np.ascontiguousarray(bias[n0:n0 + NL])
        in_maps.append({"x": x, "qw": qw_s, "st": s_s, "zt": z_s, "bias": b_s})
    return in_maps


_CACHED_NC = None


def kernel(x, qweight, wscales, wzeros, bias):
    global _CACHED_NC
    x = np.asarray(x, dtype=ml_dtypes.bfloat16)
    qweight = np.asarray(qweight, dtype=np.int32)
    wscales = np.asarray(wscales, dtype=ml_dtypes.bfloat16)
    wzeros = np.asarray(wzeros, dtype=ml_dtypes.bfloat16)
    bias = np.asarray(bias, dtype=ml_dtypes.bfloat16)

    if _CACHED_NC is None:
        _CACHED_NC = build_nc(M_FULL)
    nc = _CACHED_NC
    in_maps = shard_inputs(x, qweight, wscales, wzeros, bias)
    res = run_bass_kernel_spmd(nc, in_maps, list(range(N_CORES)))
    outs = [res.results[i]["out"] for i in range(N_CORES)]
    return np.concatenate(outs, axis=1)



# revision 14
# speedup vs baseline: 1.0231x; 1.0231x over previous
"""AWQ W4A16 Linear (out = x @ dequant(qweight) + bias) on 8 TRN2 NeuronCores.

Tensor-parallel over out_features: each core owns a contiguous slice of
N = 12288 (1536 columns), dequantizes its int4 weight shard on-chip into a
SBUF-resident bf16 [K, N_local] matrix, and runs a PE-bound matmul over the
replicated activations. No collectives; the host concatenates the 8 column
slices.

Production path (v2, build_nc_v2): the host pre-unpacks the int4 nibbles to
bf16 values and lays x.T and the unpacked q out in a (j, g, kt) k-order so
that partition p of every k-tile sees one quant group (g = p % 64).  Scale
and zero SBUF tiles are therefore kt-invariant, and dequant is two DVE
tensor_tensor ops per k-tile writing W' straight into its matmul layout —
no DRAM staging round-trip.  Matmuls are kt-outer/nb-inner so 3 consecutive
matmuls share the stationary lhsT; 8 psum banks, bias added on GPSIMD, the
psum->bf16 eviction on ACT.  qb/x loads ride the ACT HWDGE ring and out
stores the SP ring so back-to-back executions don't fence on a shared DMA
FIFO.  The v1 path (DRAM-staged dequant with xbar-transpose loads) is kept
for A/B timing under phases="all"/"mm"/"deq".
"""

import numpy as np
import ml_dtypes
from contextlib import ExitStack

import concourse.bass as bass
import concourse.bacc as bacc
import concourse.mybir as mybir
import concourse.tile as tile
from concourse.bass_utils import run_bass_kernel_spmd

BF16 = mybir.dt.bfloat16
I32 = mybir.dt.int32
F32 = mybir.dt.float32

M_FULL = 4096
K = 4096
N_FULL = 12288
N_CORES = 8
NL = N_FULL // N_CORES          # 1536 out features per core
GS = 64                         # quant group size
NG = K // GS                    # 64 groups
NKT = K // 128                  # 32 k-tiles
PANEL = 512                     # m-panel size
NB = NL // 512                  # 3 psum n-blocks per core
# dequant k-chunks (pipelines W' availability); graduated so the first
# W' tiles reach the PE quickly: sizes are in k-tiles (128 k each)
CH_KT = [2, 2, 4, 8, 8, 8]
NCH = len(CH_KT)
CH_KT0 = [sum(CH_KT[:i]) for i in range(NCH)]   # start k-tile per chunk


def build_nc(m: int = M_FULL, phases: str = "all",
             repeat: int = 1, debug_taps: bool = False) -> bass.Bass:
    if phases.startswith("v2"):
        return build_nc_v2(m, phases, repeat)
    nc = bacc.Bacc(None)
    x = nc.dram_tensor("x", [m, K], BF16, kind="ExternalInput")
    qw = nc.dram_tensor("qw", [NL // 4, K // 2], I32, kind="ExternalInput")
    # st/zt[t, r, g] = wscales/wzeros[g, n0 + 4*t + r]  (host pre-arranged)
    st = nc.dram_tensor("st", [NL // 4, 4, NG], BF16, kind="ExternalInput")
    zt = nc.dram_tensor("zt", [NL // 4, 4, NG], I32, kind="ExternalInput")
    bias = nc.dram_tensor("bias", [NL], BF16, kind="ExternalInput")
    out = nc.dram_tensor("out", [m, NL], BF16, kind="ExternalOutput")

    n_mp = m // PANEL
    n_ms = PANEL // 128

    with tile.TileContext(nc) as tc, ExitStack() as ctx:
        dram = ctx.enter_context(tc.tile_pool(name="dram", bufs=1, space="DRAM"))
        # one DRAM staging tile per k-chunk so Tile's per-tile dependency
        # tracking lets W' k-tiles of finished chunks load early
        wpre_ch = []
        for ch in range(NCH):
            w_c = dram.tile([NL, CH_KT[ch] * 128], BF16, name=f"wpre{ch}",
                            uniquify=False)
            wpre_ch.append(w_c)

        const = ctx.enter_context(tc.tile_pool(name="const", bufs=1))
        bias_sb = const.tile([128, NL], BF16)
        bias_bc = bass.AP(
            tensor=bias[:].tensor, offset=bias[:].offset, ap=[[0, 128], [1, NL]]
        )
        nc.gpsimd.dma_start(out=bias_sb[:], in_=bias_bc)

        st_all = const.tile([128, 3, 4, NG], BF16)
        zt_all = const.tile([128, 3, 4, NG], I32)
        nc.sync.dma_start(
            out=st_all[:], in_=st[:].rearrange("(t3 p) r g -> p t3 r g", p=128))
        nc.sync.dma_start(
            out=zt_all[:], in_=zt[:].rearrange("(t3 p) r g -> p t3 r g", p=128))
        st_sb = [st_all[:, t3] for t3 in range(3)]
        zt_sb = [zt_all[:, t3] for t3 in range(3)]

        # ---- pipeline body ----
        for rep in range(repeat):
            _build_pipeline(nc, tc, qw, x, out, wpre_ch, st_sb, zt_sb, bias_sb,
                            m, n_mp, n_ms, phases)
    nc.compile()
    return nc


def _build_pipeline(nc, tc, qw, x, out, wpre_ch, st_sb, zt_sb, bias_sb,
                    m, n_mp, n_ms, phases):
      with ExitStack() as ctx:
        deq = ctx.enter_context(tc.tile_pool(name="deq", bufs=2))
        qwp = ctx.enter_context(tc.tile_pool(name="qwp", bufs=1))
        wprep = ctx.enter_context(tc.tile_pool(name="wprep", bufs=2))
        wqp = ctx.enter_context(tc.tile_pool(name="wqp", bufs=1))
        xpp = ctx.enter_context(tc.tile_pool(name="xpp", bufs=2))
        psp = ctx.enter_context(tc.tile_pool(name="psp", bufs=8, space="PSUM"))
        outp = ctx.enter_context(tc.tile_pool(name="outp", bufs=2))

        do_deq = phases in ("all", "deq")
        do_mm = phases in ("all", "mm", "mm1")
        # mm1: nb-outer/kt-inner — stationary lhsT changes EVERY matmul
        # (3072 LDWEIGHTS vs 1024). Equal FLOPs/instructions; timing this
        # against "mm" isolates the exposed LDWEIGHTS cost.
        ldw_heavy = phases == "mm1"

        # x panel transpose-loads: issued on the ACT HWDGE ring so they are
        # not stuck behind the dequant staging traffic on the SP ring.
        # Only panel 0 is queued upfront; panel i+1 is queued when panel i's
        # matmuls are emitted, so early x traffic doesn't delay the first
        # W' chunk on the shared DMA engines.
        xp_tiles = []

        def load_panel(mp):
            xp_t = xpp.tile([128, NKT, PANEL], BF16, tag="xp", name=f"xp{mp}")
            # whole panel in one xbar-transpose DMA:
            # [PANEL, K] -> [128, NKT, PANEL] (out[:, e, :] = cols 128e..)
            nc.scalar.dma_start(
                out=xp_t[:],
                in_=x[mp * PANEL:(mp + 1) * PANEL, :],
                transpose=True,
            )
            xp_tiles.append(xp_t)

        if do_mm:
            load_panel(0)

        wq = [None] * NKT
        if do_deq:
            for ch in range(NCH):
                ich = CH_KT[ch] * 64        # packed int32 cols in this chunk
                i0 = CH_KT0[ch] * 64
                gch = CH_KT[ch] * 2         # 64-k groups in this chunk
                g0 = CH_KT0[ch] * 2
                # qweight shard k-chunk in one DMA: [384, ich] -> [128, 3, ich]
                qw_full = qwp.tile([128, 3, max(CH_KT) * 64], I32, tag="qw",
                                   name=f"qwb{ch}")
                qw_big = qw_full[:, :, :ich]
                nc.sync.dma_start(
                    out=qw_big,
                    in_=qw[:, i0:i0 + ich].rearrange("(t3 p) i -> p t3 i", p=128),
                )
                # wpre_ch[ch] viewed so row n = 4*t + r is addressed [r, t]
                w_rt = wpre_ch[ch][:].rearrange("(t four) k -> four t k", four=4)
                for r in range(4):
                    for t3 in range(3):
                        wp_full = wprep.tile([128, max(CH_KT) * 128], BF16,
                                             tag="wp", name=f"wp{ch}_{r}_{t3}")
                        wp_t = wp_full[:, :CH_KT[ch] * 128]
                        for c in range(2):
                            j = 2 * r + c
                            nib_full = deq.tile([128, max(CH_KT) * 64], I32,
                                                tag="nib", name=f"nib{ch}_{j}")
                            nib = nib_full[:, :ich]
                            nc.vector.tensor_scalar(
                                nib,
                                qw_big[:, t3, :],
                                4 * j, 0xF,
                                mybir.AluOpType.logical_shift_right,
                                mybir.AluOpType.bitwise_and,
                            )
                            diff_full = deq.tile([128, max(CH_KT) * 64], BF16,
                                                 tag="diff", name=f"diff{ch}_{j}")
                            diff = diff_full[:, :ich]
                            nib_g = nib.rearrange("p (g q) -> p g q", q=GS // 2)
                            diff_g = diff.rearrange("p (g q) -> p g q", q=GS // 2)
                            z_bc = zt_sb[t3][
                                :, r, g0:g0 + gch, None
                            ].broadcast_to([128, gch, GS // 2])
                            s_bc = st_sb[t3][
                                :, r, g0:g0 + gch, None
                            ].broadcast_to([128, gch, GS // 2])
                            nc.vector.tensor_tensor(
                                diff_g, nib_g, z_bc, mybir.AluOpType.subtract
                            )
                            # k_local = GS*g + 2*u + c
                            wp_view = wp_t.rearrange(
                                "p (g u two) -> p two g u", two=2, u=GS // 2
                            )[:, c]
                            nc.vector.tensor_tensor(
                                wp_view, diff_g, s_bc, mybir.AluOpType.mult
                            )
                        nc.sync.dma_start(
                            out=w_rt[r, t3 * 128:(t3 + 1) * 128], in_=wp_t
                        )
                if do_mm:
                    # all W' k-tiles of this chunk in ONE transpose DMA,
                    # right behind the chunk's stores on the SP ring
                    w_t = wqp.tile([128, CH_KT[ch], NL], BF16, tag=f"wq{ch}",
                                   name=f"wqc{ch}")
                    nc.sync.dma_start(
                        out=w_t[:], in_=wpre_ch[ch][:], transpose=True
                    )
                    for kt in range(CH_KT0[ch], CH_KT0[ch] + CH_KT[ch]):
                        wq[kt] = w_t[:, kt - CH_KT0[ch]]
        elif do_mm:
            for ch in range(NCH):
                w_t = wqp.tile([128, CH_KT[ch], NL], BF16, tag=f"wq{ch}",
                               name=f"wqc{ch}")
                nc.sync.dma_start(
                    out=w_t[:], in_=wpre_ch[ch][:], transpose=True
                )
                for kt in range(CH_KT0[ch], CH_KT0[ch] + CH_KT[ch]):
                    wq[kt] = w_t[:, kt - CH_KT0[ch]]

        if not do_mm:
            return
        for mp in range(n_mp):
            if mp + 1 < n_mp:
                load_panel(mp + 1)
            xp_t = xp_tiles[mp]
            for ms in range(n_ms):
                out_t = outp.tile([128, NL], BF16, tag="out")
                pss = [psp.tile([128, 512], F32, tag="ps", name=f"ps{nb}")
                       for nb in range(NB)]
                # kt outer / nb inner: 3 consecutive matmuls share the same
                # stationary lhsT (the PE skips redundant weight reloads)
                if ldw_heavy:
                    for nb in range(NB):
                        for kt in range(NKT):
                            nc.tensor.matmul(
                                pss[nb][:],
                                lhsT=xp_t[:, kt, ms * 128:(ms + 1) * 128],
                                rhs=wq[kt][:, nb * 512:(nb + 1) * 512],
                                start=(kt == 0),
                                stop=(kt == NKT - 1),
                            )
                else:
                    for kt in range(NKT):
                        for nb in range(NB):
                            nc.tensor.matmul(
                                pss[nb][:],
                                lhsT=xp_t[:, kt, ms * 128:(ms + 1) * 128],
                                rhs=wq[kt][:, nb * 512:(nb + 1) * 512],
                                start=(kt == 0),
                                stop=(kt == NKT - 1),
                            )
                for nb in range(NB):
                    o_slice = out_t[:, nb * 512:(nb + 1) * 512]
                    # psum -> sbuf bf16 cast on the (otherwise idle) ACT engine
                    nc.scalar.activation(
                        o_slice, pss[nb][:], mybir.ActivationFunctionType.Copy
                    )
                    nc.vector.tensor_tensor(
                        o_slice, o_slice,
                        bias_sb[:, nb * 512:(nb + 1) * 512],
                        mybir.AluOpType.add,
                    )
                nc.sync.dma_start(
                    out=out[mp * PANEL + ms * 128:mp * PANEL + (ms + 1) * 128],
                    in_=out_t[:],
                )


def build_nc_v2(m: int = M_FULL, phases: str = "v2", repeat: int = 1) -> bass.Bass:
    """v2: k-major dequant straight into SBUF, no DRAM staging.

    Host pre-arranges (see shard_inputs_v2):
      xt [K, M]  = x.T with k rows permuted to (j, g, kt) order
      qb [K, NL] = unpacked int4 values (bf16) in the same k order
      sg/zg [64, NL] = wscales/wzeros shards (natural layout)
    k-order: row r = 32*p + kt maps to k = 64*g + 32*j + kt with p = g + 64*j,
    so partition p of every k-tile sees a single quant group g = p % 64.
    Scale/zero SBUF tiles [128, NL] are therefore kt-invariant: row p holds
    sg[p % 64, :] (two plain DMA copies, no per-kt broadcast).
    Per kt: one strided qb load + two DVE tensor_tensor ops produce wq[kt]
    [128, NL] in matmul layout. Bias is preloaded into PSUM (matmuls run
    start=False), so the DVE does nothing on the output path and the next
    rep's dequant pipelines into this rep's matmul tail.
    """
    nc = bacc.Bacc(None)
    xt = nc.dram_tensor("xt", [K, m], BF16, kind="ExternalInput")
    qb = nc.dram_tensor("qb", [K, NL], BF16, kind="ExternalInput")
    sg = nc.dram_tensor("sg", [NG, NL], BF16, kind="ExternalInput")
    zg = nc.dram_tensor("zg", [NG, NL], BF16, kind="ExternalInput")
    bias = nc.dram_tensor("bias", [NL], BF16, kind="ExternalInput")
    out = nc.dram_tensor("out", [m, NL], BF16, kind="ExternalOutput")

    n_mp = m // PANEL
    n_ms = PANEL // 128

    with tile.TileContext(nc) as tc, ExitStack() as ctx:
        const = ctx.enter_context(tc.tile_pool(name="const", bufs=1))
        bias_sb = const.tile([128, NL], BF16)
        bias_bc = bass.AP(
            tensor=bias[:].tensor, offset=bias[:].offset, ap=[[0, 128], [1, NL]]
        )
        nc.gpsimd.dma_start(out=bias_sb[:], in_=bias_bc)

        # s_bc/z_bc [128, NL]: partitions 0-63 and 64-127 both hold rows
        # 0..63 of sg/zg (partition p <-> group p % 64)
        s_bc = const.tile([128, NL], BF16)
        z_bc = const.tile([128, NL], BF16)
        for half in range(2):
            nc.sync.dma_start(
                out=s_bc[:].rearrange("(j g) n -> j g n", j=2)[half], in_=sg[:])
            nc.sync.dma_start(
                out=z_bc[:].rearrange("(j g) n -> j g n", j=2)[half], in_=zg[:])

        for rep in range(repeat):
            _build_pipeline_v2(nc, tc, xt, qb, out, s_bc, z_bc, bias_sb,
                               m, n_mp, n_ms, phases)
    nc.compile()
    return nc


def _build_pipeline_v2(nc, tc, xt, qb, out, s_bc, z_bc, bias_sb,
                       m, n_mp, n_ms, phases):
    with ExitStack() as ctx:
        qkp = ctx.enter_context(tc.tile_pool(name="qkp", bufs=4))
        wqp = ctx.enter_context(tc.tile_pool(name="wqp", bufs=1))
        xpp = ctx.enter_context(tc.tile_pool(name="xpp", bufs=2))
        psp = ctx.enter_context(tc.tile_pool(name="psp", bufs=8, space="PSUM"))
        outp = ctx.enter_context(tc.tile_pool(name="outp", bufs=2))

        do_deq = phases in ("v2", "v2deq")
        do_mm = phases in ("v2", "v2mm")

        xp_tiles = []

        def load_panel(mp):
            if mp == 0:
                # first panel in 128-row sub-tiles so the first matmuls
                # start after ~1 MB of x traffic instead of 4 MB
                subs = []
                for ms in range(n_ms):
                    m0 = mp * PANEL + ms * 128
                    sub = xpp.tile([128, NKT, 128], BF16, tag="xp",
                                   name=f"xp0_{ms}")
                    nc.scalar.dma_start(
                        out=sub[:],
                        in_=xt[:, m0:m0 + 128].rearrange(
                            "(p kt) m -> p kt m", kt=NKT),
                    )
                    subs.append(sub)
                xp_tiles.append(subs)
                return
            xp_t = xpp.tile([128, NKT, PANEL], BF16, tag="xp", name=f"xp{mp}")
            nc.scalar.dma_start(
                out=xp_t[:],
                in_=xt[:, mp * PANEL:(mp + 1) * PANEL].rearrange(
                    "(p kt) m -> p kt m", kt=NKT),
            )
            xp_tiles.append(xp_t)

        if do_mm:
            load_panel(0)

        wq = [None] * NKT
        qb_r = qb[:].rearrange("(p kt) n -> p kt n", kt=NKT)
        for kt in range(NKT):
            w_t = wqp.tile([128, NL], BF16, tag=f"wq{kt}", name=f"wqt{kt}")
            wq[kt] = w_t[:]
            if do_deq:
                qk = qkp.tile([128, NL], BF16, tag="qk", name=f"qk{kt}")
                # ACT HWDGE ring: the SP ring carries the out stores, whose
                # last members only become runnable at rep end — queueing qb
                # behind them would fence the next rep's dequant chain
                nc.scalar.dma_start(out=qk[:], in_=qb_r[:, kt])
                nc.vector.tensor_tensor(
                    w_t[:], qk[:], z_bc[:], mybir.AluOpType.subtract)
                nc.vector.tensor_tensor(
                    w_t[:], w_t[:], s_bc[:], mybir.AluOpType.mult)

        if not do_mm:
            return
        for mp in range(n_mp):
            if mp + 1 < n_mp:
                load_panel(mp + 1)
            xp_t = xp_tiles[mp]
            for ms in range(n_ms):
                if mp == 0:
                    x_ms = xp_t[ms][:, :, 0:128]
                else:
                    x_ms = xp_t[:, :, ms * 128:(ms + 1) * 128]
                out_t = outp.tile([128, NL], BF16, tag="out")
                pss = [psp.tile([128, 512], F32, tag="ps", name=f"ps{nb}")
                       for nb in range(NB)]
                for kt in range(NKT):
                    for nb in range(NB):
                        nc.tensor.matmul(
                            pss[nb][:],
                            lhsT=x_ms[:, kt],
                            rhs=wq[kt][:, nb * 512:(nb + 1) * 512],
                            start=(kt == 0),
                            stop=(kt == NKT - 1),
                        )
                for nb in range(NB):
                    o_slice = out_t[:, nb * 512:(nb + 1) * 512]
                    nc.scalar.activation(
                        o_slice, pss[nb][:],
                        mybir.ActivationFunctionType.Copy,
                    )
                    # bias add on GPSIMD: keeps the DVE queue pure-dequant so
                    # the next rep's dequant isn't fenced behind this rep's
                    # output tail (DVE is in-order)
                    nc.gpsimd.tensor_tensor(
                        o_slice, o_slice,
                        bias_sb[:, nb * 512:(nb + 1) * 512],
                        mybir.AluOpType.add,
                    )
                nc.sync.dma_start(
                    out=out[mp * PANEL + ms * 128:mp * PANEL + (ms + 1) * 128],
                    in_=out_t[:],
                )


def _perm_k_rows(a):
    """Reorder axis-0 (length K) from natural to (j, g, kt) order."""
    rest = a.shape[1:]
    return np.ascontiguousarray(
        a.reshape(NG, 2, NKT, *rest).transpose(1, 0, 2, 3)
        .reshape(K, *rest))


def _unpack_q(qw_s):
    """[NL//4, K//2] int32 -> [K, NL] int4 values, k-major natural order."""
    shifts = (4 * np.arange(8, dtype=np.int32)).reshape(1, 1, 8)
    nib = (qw_s[:, :, None] >> shifts) & 0xF            # [NL//4, K//2, 8]
    nib = nib.reshape(NL // 4, K // 2, 4, 2)
    nib = nib.transpose(0, 2, 1, 3).reshape(NL, K)      # [NL, K]
    return np.ascontiguousarray(nib.T)                  # [K, NL]


def shard_inputs_v2(x, qweight, wscales, wzeros, bias):
    xt = _perm_k_rows(np.ascontiguousarray(x.T))
    in_maps = []
    for i in range(N_CORES):
        n0 = i * NL
        qb = _perm_k_rows(_unpack_q(
            np.ascontiguousarray(qweight[n0 // 4:(n0 + NL) // 4]))
        ).astype(ml_dtypes.bfloat16)
        s_s = np.ascontiguousarray(wscales[:, n0:n0 + NL])
        z_s = np.ascontiguousarray(wzeros[:, n0:n0 + NL])
        b_s = np.ascontiguousarray(bias[n0:n0 + NL])
        in_maps.append({"xt": xt, "qb": qb, "sg": s_s, "zg": z_s, "bias": b_s})
    return in_maps


def shard_inputs(x, qweight, wscales, wzeros, bias):
    """Split the full problem into per-core input maps."""
    in_maps = []
    x = np.ascontiguousarray(x)
    for i in range(N_CORES):
        n0 = i * NL
        qw_s = np.ascontiguousarray(qweight[n0 // 4:(n0 + NL) // 4])
        s_s = np.ascontiguousarray(wscales[:, n0:n0 + NL].T).reshape(NL // 4, 4, NG)
        z_s = np.ascontiguousarray(
            wzeros[:, n0:n0 + NL].T.astype(np.int32)).reshape(NL // 4, 4, NG)
        b_s = np.ascontiguousarray(bias[n0:n0 + NL])
        in_maps.append({"x": x, "qw": qw_s, "st": s_s, "zt": z_s, "bias": b_s})
    return in_maps


_CACHED_NC = None


def kernel(x, qweight, wscales, wzeros, bias):
    global _CACHED_NC
    x = np.asarray(x, dtype=ml_dtypes.bfloat16)
    qweight = np.asarray(qweight, dtype=np.int32)
    wscales = np.asarray(wscales, dtype=ml_dtypes.bfloat16)
    wzeros = np.asarray(wzeros, dtype=ml_dtypes.bfloat16)
    bias = np.asarray(bias, dtype=ml_dtypes.bfloat16)

    if _CACHED_NC is None:
        _CACHED_NC = build_nc_v2(M_FULL)
    nc = _CACHED_NC
    in_maps = shard_inputs_v2(x, qweight, wscales, wzeros, bias)
    res = run_bass_kernel_spmd(nc, in_maps, list(range(N_CORES)))
    outs = [res.results[i]["out"] for i in range(N_CORES)]
    return np.concatenate(outs, axis=1)



# revision 17
# speedup vs baseline: 1.0233x; 1.0002x over previous
"""AWQ W4A16 Linear (out = x @ dequant(qweight) + bias) on 8 TRN2 NeuronCores.

Tensor-parallel over out_features: each core owns a contiguous slice of
N = 12288 (1536 columns), dequantizes its int4 weight shard on-chip into a
SBUF-resident bf16 [K, N_local] matrix, and runs a PE-bound matmul over the
replicated activations. No collectives; the host concatenates the 8 column
slices.

Production path (v2, build_nc_v2): the host pre-unpacks the int4 nibbles to
bf16 values and lays x.T and the unpacked q out in a (j, g, kt) k-order so
that partition p of every k-tile sees one quant group (g = p % 64).  Scale
and zero SBUF tiles are therefore kt-invariant, and dequant is two DVE
tensor_tensor ops per k-tile writing W' straight into its matmul layout —
no DRAM staging round-trip.  Matmuls are kt-outer/nb-inner so 3 consecutive
matmuls share the stationary lhsT; 8 psum banks, bias added on GPSIMD, the
psum->bf16 eviction on ACT.  qb/x loads ride the ACT HWDGE ring and out
stores the SP ring so back-to-back executions don't fence on a shared DMA
FIFO.  The v1 path (DRAM-staged dequant with xbar-transpose loads) is kept
for A/B timing under phases="all"/"mm"/"deq".
"""

import numpy as np
import ml_dtypes
from contextlib import ExitStack

import concourse.bass as bass
import concourse.bacc as bacc
import concourse.mybir as mybir
import concourse.tile as tile
from concourse.bass_utils import run_bass_kernel_spmd

BF16 = mybir.dt.bfloat16
I32 = mybir.dt.int32
F32 = mybir.dt.float32

M_FULL = 4096
K = 4096
N_FULL = 12288
N_CORES = 8
NL = N_FULL // N_CORES          # 1536 out features per core
GS = 64                         # quant group size
NG = K // GS                    # 64 groups
NKT = K // 128                  # 32 k-tiles
PANEL = 512                     # m-panel size
NB = NL // 512                  # 3 psum n-blocks per core
# dequant k-chunks (pipelines W' availability); graduated so the first
# W' tiles reach the PE quickly: sizes are in k-tiles (128 k each)
CH_KT = [2, 2, 4, 8, 8, 8]
NCH = len(CH_KT)
CH_KT0 = [sum(CH_KT[:i]) for i in range(NCH)]   # start k-tile per chunk


def build_nc(m: int = M_FULL, phases: str = "all",
             repeat: int = 1, debug_taps: bool = False) -> bass.Bass:
    if phases.startswith("v2"):
        return build_nc_v2(m, phases, repeat)
    nc = bacc.Bacc(None)
    x = nc.dram_tensor("x", [m, K], BF16, kind="ExternalInput")
    qw = nc.dram_tensor("qw", [NL // 4, K // 2], I32, kind="ExternalInput")
    # st/zt[t, r, g] = wscales/wzeros[g, n0 + 4*t + r]  (host pre-arranged)
    st = nc.dram_tensor("st", [NL // 4, 4, NG], BF16, kind="ExternalInput")
    zt = nc.dram_tensor("zt", [NL // 4, 4, NG], I32, kind="ExternalInput")
    bias = nc.dram_tensor("bias", [NL], BF16, kind="ExternalInput")
    out = nc.dram_tensor("out", [m, NL], BF16, kind="ExternalOutput")

    n_mp = m // PANEL
    n_ms = PANEL // 128

    with tile.TileContext(nc) as tc, ExitStack() as ctx:
        dram = ctx.enter_context(tc.tile_pool(name="dram", bufs=1, space="DRAM"))
        # one DRAM staging tile per k-chunk so Tile's per-tile dependency
        # tracking lets W' k-tiles of finished chunks load early
        wpre_ch = []
        for ch in range(NCH):
            w_c = dram.tile([NL, CH_KT[ch] * 128], BF16, name=f"wpre{ch}",
                            uniquify=False)
            wpre_ch.append(w_c)

        const = ctx.enter_context(tc.tile_pool(name="const", bufs=1))
        bias_sb = const.tile([128, NL], BF16)
        bias_bc = bass.AP(
            tensor=bias[:].tensor, offset=bias[:].offset, ap=[[0, 128], [1, NL]]
        )
        nc.gpsimd.dma_start(out=bias_sb[:], in_=bias_bc)

        st_all = const.tile([128, 3, 4, NG], BF16)
        zt_all = const.tile([128, 3, 4, NG], I32)
        nc.sync.dma_start(
            out=st_all[:], in_=st[:].rearrange("(t3 p) r g -> p t3 r g", p=128))
        nc.sync.dma_start(
            out=zt_all[:], in_=zt[:].rearrange("(t3 p) r g -> p t3 r g", p=128))
        st_sb = [st_all[:, t3] for t3 in range(3)]
        zt_sb = [zt_all[:, t3] for t3 in range(3)]

        # ---- pipeline body ----
        for rep in range(repeat):
            _build_pipeline(nc, tc, qw, x, out, wpre_ch, st_sb, zt_sb, bias_sb,
                            m, n_mp, n_ms, phases)
    nc.compile()
    return nc


def _build_pipeline(nc, tc, qw, x, out, wpre_ch, st_sb, zt_sb, bias_sb,
                    m, n_mp, n_ms, phases):
      with ExitStack() as ctx:
        deq = ctx.enter_context(tc.tile_pool(name="deq", bufs=2))
        qwp = ctx.enter_context(tc.tile_pool(name="qwp", bufs=1))
        wprep = ctx.enter_context(tc.tile_pool(name="wprep", bufs=2))
        wqp = ctx.enter_context(tc.tile_pool(name="wqp", bufs=1))
        xpp = ctx.enter_context(tc.tile_pool(name="xpp", bufs=2))
        psp = ctx.enter_context(tc.tile_pool(name="psp", bufs=8, space="PSUM"))
        outp = ctx.enter_context(tc.tile_pool(name="outp", bufs=2))

        do_deq = phases in ("all", "deq")
        do_mm = phases in ("all", "mm", "mm1")
        # mm1: nb-outer/kt-inner — stationary lhsT changes EVERY matmul
        # (3072 LDWEIGHTS vs 1024). Equal FLOPs/instructions; timing this
        # against "mm" isolates the exposed LDWEIGHTS cost.
        ldw_heavy = phases == "mm1"

        # x panel transpose-loads: issued on the ACT HWDGE ring so they are
        # not stuck behind the dequant staging traffic on the SP ring.
        # Only panel 0 is queued upfront; panel i+1 is queued when panel i's
        # matmuls are emitted, so early x traffic doesn't delay the first
        # W' chunk on the shared DMA engines.
        xp_tiles = []

        def load_panel(mp):
            xp_t = xpp.tile([128, NKT, PANEL], BF16, tag="xp", name=f"xp{mp}")
            # whole panel in one xbar-transpose DMA:
            # [PANEL, K] -> [128, NKT, PANEL] (out[:, e, :] = cols 128e..)
            nc.scalar.dma_start(
                out=xp_t[:],
                in_=x[mp * PANEL:(mp + 1) * PANEL, :],
                transpose=True,
            )
            xp_tiles.append(xp_t)

        if do_mm:
            load_panel(0)

        wq = [None] * NKT
        if do_deq:
            for ch in range(NCH):
                ich = CH_KT[ch] * 64        # packed int32 cols in this chunk
                i0 = CH_KT0[ch] * 64
                gch = CH_KT[ch] * 2         # 64-k groups in this chunk
                g0 = CH_KT0[ch] * 2
                # qweight shard k-chunk in one DMA: [384, ich] -> [128, 3, ich]
                qw_full = qwp.tile([128, 3, max(CH_KT) * 64], I32, tag="qw",
                                   name=f"qwb{ch}")
                qw_big = qw_full[:, :, :ich]
                nc.sync.dma_start(
                    out=qw_big,
                    in_=qw[:, i0:i0 + ich].rearrange("(t3 p) i -> p t3 i", p=128),
                )
                # wpre_ch[ch] viewed so row n = 4*t + r is addressed [r, t]
                w_rt = wpre_ch[ch][:].rearrange("(t four) k -> four t k", four=4)
                for r in range(4):
                    for t3 in range(3):
                        wp_full = wprep.tile([128, max(CH_KT) * 128], BF16,
                                             tag="wp", name=f"wp{ch}_{r}_{t3}")
                        wp_t = wp_full[:, :CH_KT[ch] * 128]
                        for c in range(2):
                            j = 2 * r + c
                            nib_full = deq.tile([128, max(CH_KT) * 64], I32,
                                                tag="nib", name=f"nib{ch}_{j}")
                            nib = nib_full[:, :ich]
                            nc.vector.tensor_scalar(
                                nib,
                                qw_big[:, t3, :],
                                4 * j, 0xF,
                                mybir.AluOpType.logical_shift_right,
                                mybir.AluOpType.bitwise_and,
                            )
                            diff_full = deq.tile([128, max(CH_KT) * 64], BF16,
                                                 tag="diff", name=f"diff{ch}_{j}")
                            diff = diff_full[:, :ich]
                            nib_g = nib.rearrange("p (g q) -> p g q", q=GS // 2)
                            diff_g = diff.rearrange("p (g q) -> p g q", q=GS // 2)
                            z_bc = zt_sb[t3][
                                :, r, g0:g0 + gch, None
                            ].broadcast_to([128, gch, GS // 2])
                            s_bc = st_sb[t3][
                                :, r, g0:g0 + gch, None
                            ].broadcast_to([128, gch, GS // 2])
                            nc.vector.tensor_tensor(
                                diff_g, nib_g, z_bc, mybir.AluOpType.subtract
                            )
                            # k_local = GS*g + 2*u + c
                            wp_view = wp_t.rearrange(
                                "p (g u two) -> p two g u", two=2, u=GS // 2
                            )[:, c]
                            nc.vector.tensor_tensor(
                                wp_view, diff_g, s_bc, mybir.AluOpType.mult
                            )
                        nc.sync.dma_start(
                            out=w_rt[r, t3 * 128:(t3 + 1) * 128], in_=wp_t
                        )
                if do_mm:
                    # all W' k-tiles of this chunk in ONE transpose DMA,
                    # right behind the chunk's stores on the SP ring
                    w_t = wqp.tile([128, CH_KT[ch], NL], BF16, tag=f"wq{ch}",
                                   name=f"wqc{ch}")
                    nc.sync.dma_start(
                        out=w_t[:], in_=wpre_ch[ch][:], transpose=True
                    )
                    for kt in range(CH_KT0[ch], CH_KT0[ch] + CH_KT[ch]):
                        wq[kt] = w_t[:, kt - CH_KT0[ch]]
        elif do_mm:
            for ch in range(NCH):
                w_t = wqp.tile([128, CH_KT[ch], NL], BF16, tag=f"wq{ch}",
                               name=f"wqc{ch}")
                nc.sync.dma_start(
                    out=w_t[:], in_=wpre_ch[ch][:], transpose=True
                )
                for kt in range(CH_KT0[ch], CH_KT0[ch] + CH_KT[ch]):
                    wq[kt] = w_t[:, kt - CH_KT0[ch]]

        if not do_mm:
            return
        for mp in range(n_mp):
            if mp + 1 < n_mp:
                load_panel(mp + 1)
            xp_t = xp_tiles[mp]
            for ms in range(n_ms):
                out_t = outp.tile([128, NL], BF16, tag="out")
                pss = [psp.tile([128, 512], F32, tag="ps", name=f"ps{nb}")
                       for nb in range(NB)]
                # kt outer / nb inner: 3 consecutive matmuls share the same
                # stationary lhsT (the PE skips redundant weight reloads)
                if ldw_heavy:
                    for nb in range(NB):
                        for kt in range(NKT):
                            nc.tensor.matmul(
                                pss[nb][:],
                                lhsT=xp_t[:, kt, ms * 128:(ms + 1) * 128],
                                rhs=wq[kt][:, nb * 512:(nb + 1) * 512],
                                start=(kt == 0),
                                stop=(kt == NKT - 1),
                            )
                else:
                    for kt in range(NKT):
                        for nb in range(NB):
                            nc.tensor.matmul(
                                pss[nb][:],
                                lhsT=xp_t[:, kt, ms * 128:(ms + 1) * 128],
                                rhs=wq[kt][:, nb * 512:(nb + 1) * 512],
                                start=(kt == 0),
                                stop=(kt == NKT - 1),
                            )
                for nb in range(NB):
                    o_slice = out_t[:, nb * 512:(nb + 1) * 512]
                    # psum -> sbuf bf16 cast on the (otherwise idle) ACT engine
                    nc.scalar.activation(
                        o_slice, pss[nb][:], mybir.ActivationFunctionType.Copy
                    )
                    nc.vector.tensor_tensor(
                        o_slice, o_slice,
                        bias_sb[:, nb * 512:(nb + 1) * 512],
                        mybir.AluOpType.add,
                    )
                nc.sync.dma_start(
                    out=out[mp * PANEL + ms * 128:mp * PANEL + (ms + 1) * 128],
                    in_=out_t[:],
                )


def build_nc_v2(m: int = M_FULL, phases: str = "v2", repeat: int = 1) -> bass.Bass:
    """v2: k-major dequant straight into SBUF, no DRAM staging.

    Host pre-arranges (see shard_inputs_v2):
      xt [K, M]  = x.T with k rows permuted to (j, g, kt) order
      qb [K, NL] = unpacked int4 values (bf16) in the same k order
      sg/zg [64, NL] = wscales/wzeros shards (natural layout)
    k-order: row r = 32*p + kt maps to k = 64*g + 32*j + kt with p = g + 64*j,
    so partition p of every k-tile sees a single quant group g = p % 64.
    Scale/zero SBUF tiles [128, NL] are therefore kt-invariant: row p holds
    sg[p % 64, :] (two plain DMA copies, no per-kt broadcast).
    Per kt: one strided qb load + two DVE tensor_tensor ops produce wq[kt]
    [128, NL] in matmul layout. Bias is preloaded into PSUM (matmuls run
    start=False), so the DVE does nothing on the output path and the next
    rep's dequant pipelines into this rep's matmul tail.
    """
    nc = bacc.Bacc(None)
    xt = nc.dram_tensor("xt", [K, m], BF16, kind="ExternalInput")
    qb = nc.dram_tensor("qb", [K, NL], BF16, kind="ExternalInput")
    sg = nc.dram_tensor("sg", [NG, NL], BF16, kind="ExternalInput")
    zg = nc.dram_tensor("zg", [NG, NL], BF16, kind="ExternalInput")
    bias = nc.dram_tensor("bias", [NL], BF16, kind="ExternalInput")
    out = nc.dram_tensor("out", [m, NL], BF16, kind="ExternalOutput")

    n_mp = m // PANEL
    n_ms = PANEL // 128

    with tile.TileContext(nc) as tc, ExitStack() as ctx:
        const = ctx.enter_context(tc.tile_pool(name="const", bufs=1))
        bias_sb = const.tile([128, NL], BF16)
        bias_bc = bass.AP(
            tensor=bias[:].tensor, offset=bias[:].offset, ap=[[0, 128], [1, NL]]
        )
        nc.gpsimd.dma_start(out=bias_sb[:], in_=bias_bc)

        # s_bc/z_bc [128, NL]: partitions 0-63 and 64-127 both hold rows
        # 0..63 of sg/zg (partition p <-> group p % 64)
        s_bc = const.tile([128, NL], BF16)
        z_bc = const.tile([128, NL], BF16)
        for half in range(2):
            nc.sync.dma_start(
                out=s_bc[:].rearrange("(j g) n -> j g n", j=2)[half], in_=sg[:])
            nc.sync.dma_start(
                out=z_bc[:].rearrange("(j g) n -> j g n", j=2)[half], in_=zg[:])

        for rep in range(repeat):
            _build_pipeline_v2(nc, tc, xt, qb, out, s_bc, z_bc, bias_sb,
                               m, n_mp, n_ms, phases)
    nc.compile()
    return nc


def _build_pipeline_v2(nc, tc, xt, qb, out, s_bc, z_bc, bias_sb,
                       m, n_mp, n_ms, phases):
    with ExitStack() as ctx:
        qkp = ctx.enter_context(tc.tile_pool(name="qkp", bufs=4))
        wqp = ctx.enter_context(tc.tile_pool(name="wqp", bufs=1))
        xpp = ctx.enter_context(tc.tile_pool(name="xpp", bufs=2))
        psp = ctx.enter_context(tc.tile_pool(name="psp", bufs=8, space="PSUM"))
        outp = ctx.enter_context(tc.tile_pool(name="outp", bufs=2))

        do_deq = phases in ("v2", "v2s", "v2deq")
        do_mm = phases in ("v2", "v2s", "v2mm")
        # v2s: out stores ride the GPSIMD/SWDGE queue instead of SP, so the
        # SP ring never holds late-runnable work that would delay the next
        # rep's qb loads queued behind it
        store_engine = nc.gpsimd if phases == "v2s" else nc.sync

        xp_tiles = []

        def load_panel(mp):
            if mp == 0:
                # first panel in 128-row sub-tiles so the first matmuls
                # start after ~1 MB of x traffic instead of 4 MB
                subs = []
                for ms in range(n_ms):
                    m0 = mp * PANEL + ms * 128
                    sub = xpp.tile([128, NKT, 128], BF16, tag="xp",
                                   name=f"xp0_{ms}")
                    nc.scalar.dma_start(
                        out=sub[:],
                        in_=xt[:, m0:m0 + 128].rearrange(
                            "(p kt) m -> p kt m", kt=NKT),
                    )
                    subs.append(sub)
                xp_tiles.append(subs)
                return
            xp_t = xpp.tile([128, NKT, PANEL], BF16, tag="xp", name=f"xp{mp}")
            nc.scalar.dma_start(
                out=xp_t[:],
                in_=xt[:, mp * PANEL:(mp + 1) * PANEL].rearrange(
                    "(p kt) m -> p kt m", kt=NKT),
            )
            xp_tiles.append(xp_t)

        if do_mm:
            load_panel(0)

        wq = [None] * NKT
        qb_r = qb[:].rearrange("(p kt) n -> p kt n", kt=NKT)
        for kt in range(NKT):
            w_t = wqp.tile([128, NL], BF16, tag=f"wq{kt}", name=f"wqt{kt}")
            wq[kt] = w_t[:]
            if do_deq:
                qk = qkp.tile([128, NL], BF16, tag="qk", name=f"qk{kt}")
                # keep qb off the ring that carries the out stores, whose
                # last members only become runnable at rep end — queueing qb
                # behind them would fence the next rep's dequant chain
                qb_dma = nc.sync if phases == "v2s" else nc.scalar
                qb_dma.dma_start(out=qk[:], in_=qb_r[:, kt])
                nc.vector.tensor_tensor(
                    w_t[:], qk[:], z_bc[:], mybir.AluOpType.subtract)
                nc.vector.tensor_tensor(
                    w_t[:], w_t[:], s_bc[:], mybir.AluOpType.mult)

        if not do_mm:
            return
        for mp in range(n_mp):
            if mp + 1 < n_mp:
                load_panel(mp + 1)
            xp_t = xp_tiles[mp]
            for ms in range(n_ms):
                if mp == 0:
                    x_ms = xp_t[ms][:, :, 0:128]
                else:
                    x_ms = xp_t[:, :, ms * 128:(ms + 1) * 128]
                out_t = outp.tile([128, NL], BF16, tag="out")
                pss = [psp.tile([128, 512], F32, tag="ps", name=f"ps{nb}")
                       for nb in range(NB)]
                for kt in range(NKT):
                    for nb in range(NB):
                        nc.tensor.matmul(
                            pss[nb][:],
                            lhsT=x_ms[:, kt],
                            rhs=wq[kt][:, nb * 512:(nb + 1) * 512],
                            start=(kt == 0),
                            stop=(kt == NKT - 1),
                        )
                for nb in range(NB):
                    o_slice = out_t[:, nb * 512:(nb + 1) * 512]
                    nc.scalar.activation(
                        o_slice, pss[nb][:],
                        mybir.ActivationFunctionType.Copy,
                    )
                    # bias add on GPSIMD: keeps the DVE queue pure-dequant so
                    # the next rep's dequant isn't fenced behind this rep's
                    # output tail (DVE is in-order)
                    nc.gpsimd.tensor_tensor(
                        o_slice, o_slice,
                        bias_sb[:, nb * 512:(nb + 1) * 512],
                        mybir.AluOpType.add,
                    )
                store_engine.dma_start(
                    out=out[mp * PANEL + ms * 128:mp * PANEL + (ms + 1) * 128],
                    in_=out_t[:],
                )


def _perm_k_rows(a):
    """Reorder axis-0 (length K) from natural to (j, g, kt) order."""
    rest = a.shape[1:]
    return np.ascontiguousarray(
        a.reshape(NG, 2, NKT, *rest).transpose(1, 0, 2, 3)
        .reshape(K, *rest))


def _unpack_q(qw_s):
    """[NL//4, K//2] int32 -> [K, NL] int4 values, k-major natural order."""
    shifts = (4 * np.arange(8, dtype=np.int32)).reshape(1, 1, 8)
    nib = (qw_s[:, :, None] >> shifts) & 0xF            # [NL//4, K//2, 8]
    nib = nib.reshape(NL // 4, K // 2, 4, 2)
    nib = nib.transpose(0, 2, 1, 3).reshape(NL, K)      # [NL, K]
    return np.ascontiguousarray(nib.T)                  # [K, NL]


def shard_inputs_v2(x, qweight, wscales, wzeros, bias):
    xt = _perm_k_rows(np.ascontiguousarray(x.T))
    in_maps = []
    for i in range(N_CORES):
        n0 = i * NL
        qb = _perm_k_rows(_unpack_q(
            np.ascontiguousarray(qweight[n0 // 4:(n0 + NL) // 4]))
        ).astype(ml_dtypes.bfloat16)
        s_s = np.ascontiguousarray(wscales[:, n0:n0 + NL])
        z_s = np.ascontiguousarray(wzeros[:, n0:n0 + NL])
        b_s = np.ascontiguousarray(bias[n0:n0 + NL])
        in_maps.append({"xt": xt, "qb": qb, "sg": s_s, "zg": z_s, "bias": b_s})
    return in_maps


def shard_inputs(x, qweight, wscales, wzeros, bias):
    """Split the full problem into per-core input maps."""
    in_maps = []
    x = np.ascontiguousarray(x)
    for i in range(N_CORES):
        n0 = i * NL
        qw_s = np.ascontiguousarray(qweight[n0 // 4:(n0 + NL) // 4])
        s_s = np.ascontiguousarray(wscales[:, n0:n0 + NL].T).reshape(NL // 4, 4, NG)
        z_s = np.ascontiguousarray(
            wzeros[:, n0:n0 + NL].T.astype(np.int32)).reshape(NL // 4, 4, NG)
        b_s = np.ascontiguousarray(bias[n0:n0 + NL])
        in_maps.append({"x": x, "qw": qw_s, "st": s_s, "zt": z_s, "bias": b_s})
    return in_maps


_CACHED_NC = None


def kernel(x, qweight, wscales, wzeros, bias):
    global _CACHED_NC
    x = np.asarray(x, dtype=ml_dtypes.bfloat16)
    qweight = np.asarray(qweight, dtype=np.int32)
    wscales = np.asarray(wscales, dtype=ml_dtypes.bfloat16)
    wzeros = np.asarray(wzeros, dtype=ml_dtypes.bfloat16)
    bias = np.asarray(bias, dtype=ml_dtypes.bfloat16)

    if _CACHED_NC is None:
        _CACHED_NC = build_nc_v2(M_FULL)
    nc = _CACHED_NC
    in_maps = shard_inputs_v2(x, qweight, wscales, wzeros, bias)
    res = run_bass_kernel_spmd(nc, in_maps, list(range(N_CORES)))
    outs = [res.results[i]["out"] for i in range(N_CORES)]
    return np.concatenate(outs, axis=1)

